# revision 7
# baseline (speedup 1.0000x reference)
"""Fused Linear + LayerNorm + residual-multiply kernel for 8 Trainium2 cores.

Computes, for full inputs x[B,1024], y[B,1024], weight[1024,1024], bias, gamma, beta:
    z  = x @ weight.T + bias
    ln = (z - mean(z)) * rsqrt(var(z) + eps) * gamma + beta     (over last dim)
    out = (ln + y) * y

Data-parallel over the batch dim: each of the 8 NeuronCores processes B/8 rows;
weight/bias/gamma/beta are replicated. No cross-core communication.

Key host-side tricks:
  * Mean elimination: z - mean(z) == x @ W''.T + b'' where W'' subtracts each
    input-column's mean over outputs (W''[o,i] = W[o,i] - mean_o W[:,i]) and
    b'' = b - mean(b).  The device never computes the mean - only sum(z''^2).
  * fp8 DoubleRow matmuls: x and W'' are quantized host-side to e4m3 (scaled
    by 8 and 128; LayerNorm is scale-invariant so only eps is adjusted).
    DoubleRow packs 2 fp8 weights per PE cell; contraction runs over
    [128 partitions x 2 k-blocks] per matmul -> half the matmul count.

Per-core pipeline (b_core = B/8 rows, P=128, D=1024, 16 row-tiles):
  - PE: per tile 8 DoubleRow matmuls (4 k-pairs x 2 PSUM halves) + 2 DoubleRow
    bias matmuls; short fp32 warmup matmuls bridge the input-staging window so
    the clock ramp (HAM) reaches 2.4 GHz before the real matmuls.
  - ScalarE: Square activation with accum_out -> sum(z''^2); Sqrt(./D + eps).
  - VectorE: reciprocal -> rstd; fused scalar_tensor_tensor
    u = (z'' * rstd) + y straight out of PSUM; half of o = u * y.
  - GpSimd: other half of o = u * y  (SBUF-only fp16).
  - DMA: w''/bias on the scalar queue; x8/y loads and out stores on the sync
    queue, one chunk (512 rows) prefetched ahead, stores batched per chunk.
    The last tile runs a half-split low-latency chain to shorten the drain.
  - Output is fp16, widened to fp32 on the host.
"""

import numpy as np
import ml_dtypes
from contextlib import ExitStack

import concourse.bass as bass
import concourse.mybir as mybir
import concourse.tile as tile
from concourse import bacc, bass_utils


P = 128
D = 1024
KT = D // P          # 8 k-blocks of 128 over the contraction dim
KP = KT // 2         # 4 DoubleRow k-pairs
OB = 512             # o-block width (one PSUM bank of fp32)
ST = 512             # rows per super-chunk
N_CORES = 8
EPS = 1e-5

SX = 8.0             # host scale on x before e4m3 quantization
SW = 128.0           # host scale on W'' before e4m3 quantization
SCALE = SX * SW
EPS_EFF = EPS * SCALE * SCALE

F32 = mybir.dt.float32
F16 = mybir.dt.float16
F8 = mybir.dt.float8e4

AF = mybir.ActivationFunctionType
OP = mybir.AluOpType
DR = mybir.MatmulPerfMode.DoubleRow

_BUILD_CACHE = {}


def _build(b_core: int, trivial_affine: bool):
    key = (b_core, trivial_affine)
    if key in _BUILD_CACHE:
        return _BUILD_CACHE[key]

    nst = b_core // ST
    tpc = ST // P
    nc = bacc.Bacc("TRN2", debug=False, num_devices=N_CORES)

    # x8 packed as [super-chunk, partition, k * b_local] (contiguous/partition)
    xt = nc.dram_tensor("xt", [nst, P, KT * ST], F8, kind="ExternalInput").ap()
    # y packed row-tile-major: [chunk, partition, tile_in_chunk, D]
    yh = nc.dram_tensor("yh", [nst, P, tpc, D], F16, kind="ExternalInput").ap()
    # W''.T packed as [k, partition, o] (contiguous per partition per k-block)
    wth = nc.dram_tensor("wth", [KT, P, D], F8, kind="ExternalInput").ap()
    # bias row pair for DoubleRow: [2, D] with row 1 = 0
    biash = nc.dram_tensor("biash", [2, D], F8, kind="ExternalInput").ap()
    if not trivial_affine:
        gamma = nc.dram_tensor("gamma", [D], F32, kind="ExternalInput").ap()
        beta = nc.dram_tensor("beta", [D], F32, kind="ExternalInput").ap()
    out = nc.dram_tensor("out", [nst, P, tpc, D], F16, kind="ExternalOutput").ap()

    with tile.TileContext(nc) as tc, ExitStack() as ctx:
        const = ctx.enter_context(tc.tile_pool(name="const", bufs=1))
        xtp = ctx.enter_context(tc.tile_pool(name="xtp", bufs=2))
        ypool = ctx.enter_context(tc.tile_pool(name="yp", bufs=2))
        upool = ctx.enter_context(tc.tile_pool(name="up", bufs=3))
        opool = ctx.enter_context(tc.tile_pool(name="op", bufs=2))
        stat = ctx.enter_context(tc.tile_pool(name="stat", bufs=10))
        psz = ctx.enter_context(tc.tile_pool(name="psz", bufs=4, space="PSUM"))

        # --- small constants (vector memsets; no DMA dependency) ---
        ones_f32 = const.tile([1, P], F32)
        nc.vector.memset(ones_f32[:], 1.0)
        warm_mov = const.tile([1, OB], F32)
        nc.vector.memset(warm_mov[:], 0.0)
        eps_sb = const.tile([P, 1], F32)
        nc.vector.memset(eps_sb[:], EPS_EFF)
        ones_sb = const.tile([1, 2, P], F8)
        nc.vector.memset(ones_sb[:], 1.0)

        # --- PE warmup: keep the clock-ramp window busy while inputs stage.
        # fp32 matmuls stream at 4 cyc/row, so a few cover the staging time.
        warm_ps = psz.tile([P, D], F32, tag="z_ps")
        for w in range(5):
            nc.tensor.matmul(
                warm_ps[:, 0:OB], ones_f32[:], warm_mov[:], start=True, stop=True
            )

        # --- constants from HBM (scalar queue; k-split so matmul t only
        # waits for the k-pairs it reads) ---
        wt_sb = const.tile([P, KT, D], F8)  # [i_local, k, o]
        nc.scalar.dma_start(out=wt_sb[:], in_=wth.rearrange("k p o -> p k o"))
        bias_sb = const.tile([1, 2, D], F8)
        nc.scalar.dma_start(out=bias_sb[:], in_=biash.unsqueeze(0))
        if not trivial_affine:
            gamma_f32 = const.tile([P, D], F32)
            nc.sync.dma_start(out=gamma_f32[:], in_=gamma.unsqueeze(0).to_broadcast([P, D]))
            gamma_sb = const.tile([P, D], F16)
            nc.scalar.activation(gamma_sb[:], gamma_f32[:], AF.Copy)
            beta_f32 = const.tile([P, D], F32)
            nc.sync.dma_start(out=beta_f32[:], in_=beta.unsqueeze(0).to_broadcast([P, D]))
            beta_sb = const.tile([P, D], F16)
            nc.scalar.activation(beta_sb[:], beta_f32[:], AF.Copy)

        nb = b_core // P

        x_chunks = {}
        y_chunks = {}
        o_chunks = {}

        def load_chunk(st, ksplit=False):
            xt_sb = xtp.tile([P, KT, ST], F8, name="xt_sb")  # [i_local, k, b]
            src = xt[st].rearrange("p (k b) -> p k b", k=KT)
            if ksplit:
                for t in range(KP):
                    nc.sync.dma_start(
                        out=xt_sb[:, 2 * t:2 * t + 2, :], in_=src[:, 2 * t:2 * t + 2, :]
                    )
            else:
                nc.sync.dma_start(out=xt_sb[:], in_=src)
            y_sb = ypool.tile([P, tpc, D], F16, name="y_sb")
            nc.sync.dma_start(out=y_sb[:], in_=yh[st])
            x_chunks[st] = xt_sb
            y_chunks[st] = y_sb
            o_chunks[st] = opool.tile([P, tpc, D], F16, name="o_sb")

        load_chunk(0)

        for bt in range(nb):
            st = bt // tpc
            j = bt % tpc
            last = bt == nb - 1
            if j == 0:
                if st >= 1:
                    # previous chunk's results are complete; store them first
                    # so later load issues on this queue don't block behind it
                    nc.sync.dma_start(out=out[st - 1], in_=o_chunks.pop(st - 1)[:])
                    x_chunks.pop(st - 1)
                    y_chunks.pop(st - 1)
                if st + 1 < nst:
                    load_chunk(st + 1)
            xt_sb = x_chunks[st]
            y_sb = y_chunks[st]
            o_sb = o_chunks[st]
            if last:
                nc.sync.dma_start(
                    out=out[st][:, 0:tpc - 1, :], in_=o_sb[:, 0:tpc - 1, :]
                )

            # --- matmuls: z'' = x @ W''.T + b'', DoubleRow fp8, PSUM fp32 ---
            z_ps = psz.tile([P, D], F32)
            if not last:
                for t in range(KP):
                    lhsT = xt_sb[:, 2 * t:2 * t + 2, bass.ts(j, P)]
                    for half in range(2):
                        nc.tensor.matmul(
                            z_ps[:, bass.ts(half, OB)],
                            lhsT,
                            wt_sb[:, 2 * t:2 * t + 2, bass.ts(half, OB)],
                            start=(t == 0),
                            stop=False,
                            perf_mode=DR,
                        )
                for half in range(2):
                    nc.tensor.matmul(
                        z_ps[:, bass.ts(half, OB)],
                        ones_sb[:],
                        bias_sb[:, :, bass.ts(half, OB)],
                        start=False,
                        stop=True,
                        perf_mode=DR,
                    )
            else:
                # last tile: half-major order so stats on half 0 overlap the
                # half-1 matmuls, shortening the end-of-kernel drain
                for half in range(2):
                    for t in range(KP):
                        nc.tensor.matmul(
                            z_ps[:, bass.ts(half, OB)],
                            xt_sb[:, 2 * t:2 * t + 2, bass.ts(j, P)],
                            wt_sb[:, 2 * t:2 * t + 2, bass.ts(half, OB)],
                            start=(t == 0),
                            stop=False,
                            perf_mode=DR,
                        )
                    nc.tensor.matmul(
                        z_ps[:, bass.ts(half, OB)],
                        ones_sb[:],
                        bias_sb[:, :, bass.ts(half, OB)],
                        start=False,
                        stop=True,
                        perf_mode=DR,
                    )

            # --- stats: rstd = 1/sqrt(sum(z''^2)/D + eps) (mean is 0) ---
            std = stat.tile([P, 1], F32)
            if not last:
                ssq = stat.tile([P, 1], F32)
                z2_scr = upool.tile([P, D], F16)
                nc.scalar.activation(z2_scr[:], z_ps[:], AF.Square, accum_out=ssq[:])
                nc.scalar.activation(
                    std[:], ssq[:], AF.Sqrt, bias=eps_sb[:], scale=1.0 / D
                )
            else:
                ssqh = stat.tile([P, 2], F32)
                z2_scr = upool.tile([P, D], F16)
                for half in range(2):
                    nc.scalar.activation(
                        z2_scr[:, bass.ts(half, OB)],
                        z_ps[:, bass.ts(half, OB)],
                        AF.Square,
                        accum_out=ssqh[:, half:half + 1],
                    )
                ssq = stat.tile([P, 1], F32)
                nc.vector.tensor_add(
                    out=ssq[:], in0=ssqh[:, 0:1], in1=ssqh[:, 1:2]
                )
                nc.scalar.activation(
                    std[:], ssq[:], AF.Sqrt, bias=eps_sb[:], scale=1.0 / D
                )
            rstd = stat.tile([P, 1], F32)
            nc.vector.reciprocal(rstd[:], std[:])

            # --- normalize + residual: u = z''*rstd + y;  o = (u) * y ---
            u_sb = upool.tile([P, D], F16)
            if trivial_affine:
                if not last:
                    nc.vector.scalar_tensor_tensor(
                        out=u_sb[:], in0=z_ps[:], scalar=rstd[:], in1=y_sb[:, j, :],
                        op0=OP.mult, op1=OP.add,
                    )
                    # split the final multiply: VectorE low half, GpSimd high
                    nc.vector.tensor_mul(
                        out=o_sb[:, j, 0:384], in0=u_sb[:, 0:384], in1=y_sb[:, j, 0:384]
                    )
                    nc.gpsimd.tensor_mul(
                        out=o_sb[:, j, 384:D], in0=u_sb[:, 384:D], in1=y_sb[:, j, 384:D]
                    )
                else:
                    for half in range(2):
                        hs = bass.ts(half, OB)
                        nc.vector.scalar_tensor_tensor(
                            out=u_sb[:, hs], in0=z_ps[:, hs], scalar=rstd[:],
                            in1=y_sb[:, j, hs], op0=OP.mult, op1=OP.add,
                        )
                        nc.vector.tensor_mul(
                            out=o_sb[:, j, hs], in0=u_sb[:, hs], in1=y_sb[:, j, hs]
                        )
                        # store each half as soon as it is ready
                        nc.sync.dma_start(
                            out=out[st][:, j:j + 1, hs],
                            in_=o_sb[:, j:j + 1, hs],
                        )
            else:
                ln_sb = upool.tile([P, D], F16)
                nc.vector.scalar_tensor_tensor(
                    out=ln_sb[:], in0=z_ps[:], scalar=rstd[:], in1=gamma_sb[:],
                    op0=OP.mult, op1=OP.mult,
                )
                nc.vector.tensor_add(out=ln_sb[:], in0=ln_sb[:], in1=beta_sb[:])
                nc.vector.tensor_add(out=u_sb[:], in0=ln_sb[:], in1=y_sb[:, j, :])
                nc.vector.tensor_mul(
                    out=o_sb[:, j, :], in0=u_sb[:], in1=y_sb[:, j, :]
                )
                if last:
                    nc.sync.dma_start(
                        out=out[st][:, j:j + 1, :], in_=o_sb[:, j:j + 1, :]
                    )


    nc.finalize()
    _BUILD_CACHE[key] = nc
    return nc


def _run(nc, in_maps, **kwargs):
    return bass_utils.run_bass_kernel_spmd(
        nc, in_maps, core_ids=list(range(N_CORES)), **kwargs
    )


def _q8(a):
    return np.ascontiguousarray(a.astype(ml_dtypes.float8_e4m3))


def _prepare(x, y, weight, bias, gamma, beta):
    x = np.asarray(x, dtype=np.float32)
    y = np.ascontiguousarray(y, dtype=np.float32)
    weight = np.asarray(weight, dtype=np.float32)
    bias = np.asarray(bias, dtype=np.float32)
    gamma = np.asarray(gamma, dtype=np.float32)
    beta = np.asarray(beta, dtype=np.float32)

    B, IN = x.shape
    assert IN == D and weight.shape == (D, D) and y.shape == (B, D)
    assert B % (N_CORES * ST) == 0
    b_core = B // N_CORES
    nst = b_core // ST
    tpc = ST // P

    trivial = bool(np.all(gamma == 1.0)) and bool(np.all(beta == 0.0))
    nc = _build(b_core, trivial)

    # recenter: subtract each input-column's mean over outputs, so the matmul
    # result is already mean-free along the LayerNorm axis
    wcc = weight - weight.mean(axis=0, keepdims=True)
    bcc = bias - bias.mean()

    # W''.T packed: wth_prep[k, p, o] = W''.T[k*P + p, o], e4m3 at scale SW
    wth_prep = _q8((SW * wcc.T).reshape(KT, P, D))
    bias_prep = np.zeros((2, D), dtype=np.float32)
    bias_prep[0] = SCALE * bcc
    bias_prep = _q8(bias_prep)

    in_maps = []
    for c in range(N_CORES):
        xs = x[c * b_core:(c + 1) * b_core]
        # x8 packed: xt_prep[st, p, k, b_local] = x.T[k*P + p, st*ST + b_local]
        xt_prep = _q8(
            (SX * xs.T).reshape(KT, P, nst, ST).transpose(2, 1, 0, 3)
        ).reshape(nst, P, KT * ST)
        ys = y[c * b_core:(c + 1) * b_core].astype(np.float16)
        # y packed [st, p, j, o] = y[st*ST + j*P + p, o]
        y_prep = np.ascontiguousarray(
            ys.reshape(nst, tpc, P, D).transpose(0, 2, 1, 3)
        )
        m = {
            "xt": xt_prep,
            "yh": y_prep,
            "wth": wth_prep,
            "biash": bias_prep,
        }
        if not trivial:
            m["gamma"] = gamma
            m["beta"] = beta
        in_maps.append(m)
    return nc, in_maps


def kernel(x, y, weight, bias, gamma, beta):
    nc, in_maps = _prepare(x, y, weight, bias, gamma, beta)
    res = _run(nc, in_maps)
    b_core = x.shape[0] // N_CORES
    nst = b_core // ST
    tpc = ST // P
    outs = []
    for r in res.results:
        o = np.asarray(r["out"])  # [nst, P, tpc, D] fp16
        outs.append(o.transpose(0, 2, 1, 3).reshape(b_core, D))
    return np.concatenate(outs, axis=0).astype(np.float32)


# revision 8
# speedup vs baseline: 1.0397x; 1.0397x over previous
"""Fused Linear + LayerNorm + residual-multiply kernel for 8 Trainium2 cores.

Computes, for full inputs x[B,1024], y[B,1024], weight[1024,1024], bias, gamma, beta:
    z  = x @ weight.T + bias
    ln = (z - mean(z)) * rsqrt(var(z) + eps) * gamma + beta     (over last dim)
    out = (ln + y) * y

Data-parallel over the batch dim: each of the 8 NeuronCores processes B/8 rows;
weight/bias/gamma/beta are replicated. No cross-core communication.

Key host-side tricks:
  * Mean elimination: z - mean(z) == x @ W''.T + b'' where W'' subtracts each
    input-column's mean over outputs (W''[o,i] = W[o,i] - mean_o W[:,i]) and
    b'' = b - mean(b).  The device never computes the mean - only sum(z''^2).
  * fp8 DoubleRow matmuls: x and W'' are quantized host-side to e4m3 (scaled
    by 8 and 128; LayerNorm is scale-invariant so only eps is adjusted).
    DoubleRow packs 2 fp8 weights per PE cell; contraction runs over
    [128 partitions x 2 k-blocks] per matmul -> half the matmul count.

Per-core pipeline (b_core = B/8 rows, P=128, D=1024, 16 row-tiles):
  - PE: per tile 8 DoubleRow matmuls (4 k-pairs x 2 PSUM halves) + 2 DoubleRow
    bias matmuls; short fp32 warmup matmuls bridge the input-staging window so
    the clock ramp (HAM) reaches 2.4 GHz before the real matmuls.
  - ScalarE: Square activation with accum_out -> sum(z''^2); Sqrt(./D + eps).
  - VectorE: reciprocal -> rstd; fused scalar_tensor_tensor
    u = (z'' * rstd) + y straight out of PSUM; half of o = u * y.
  - GpSimd: other half of o = u * y  (SBUF-only fp16).
  - DMA: w''/bias on the scalar queue; x8/y loads and out stores on the sync
    queue, one chunk (512 rows) prefetched ahead, stores batched per chunk.
    The last tile runs a half-split low-latency chain to shorten the drain.
  - Output is fp16, widened to fp32 on the host.
"""

import numpy as np
import ml_dtypes
from contextlib import ExitStack

import concourse.bass as bass
import concourse.mybir as mybir
import concourse.tile as tile
from concourse import bacc, bass_utils


P = 128
D = 1024
KT = D // P          # 8 k-blocks of 128 over the contraction dim
KP = KT // 2         # 4 DoubleRow k-pairs
OB = 512             # o-block width (one PSUM bank of fp32)
ST = 512             # rows per super-chunk
N_CORES = 8
EPS = 1e-5

SX = 8.0             # host scale on x before e4m3 quantization
SW = 128.0           # host scale on W'' before e4m3 quantization
SCALE = SX * SW
EPS_EFF = EPS * SCALE * SCALE

F32 = mybir.dt.float32
F16 = mybir.dt.float16
F8 = mybir.dt.float8e4

AF = mybir.ActivationFunctionType
OP = mybir.AluOpType
DR = mybir.MatmulPerfMode.DoubleRow

_BUILD_CACHE = {}


def _build(b_core: int, trivial_affine: bool):
    key = (b_core, trivial_affine)
    if key in _BUILD_CACHE:
        return _BUILD_CACHE[key]

    nst = b_core // ST
    tpc = ST // P
    nc = bacc.Bacc("TRN2", debug=False, num_devices=N_CORES)

    # x8 packed as [super-chunk, partition, k * b_local] (contiguous/partition)
    xt = nc.dram_tensor("xt", [nst, P, KT * ST], F8, kind="ExternalInput").ap()
    # y packed row-tile-major: [chunk, partition, tile_in_chunk, D]
    yh = nc.dram_tensor("yh", [nst, P, tpc, D], F16, kind="ExternalInput").ap()
    # W''.T packed as [k, partition, o] (contiguous per partition per k-block)
    wth = nc.dram_tensor("wth", [KT, P, D], F8, kind="ExternalInput").ap()
    # bias row pair for DoubleRow: [2, D] with row 1 = 0
    biash = nc.dram_tensor("biash", [2, D], F8, kind="ExternalInput").ap()
    if not trivial_affine:
        gamma = nc.dram_tensor("gamma", [D], F32, kind="ExternalInput").ap()
        beta = nc.dram_tensor("beta", [D], F32, kind="ExternalInput").ap()
    out = nc.dram_tensor("out", [nst, P, tpc, D], F16, kind="ExternalOutput").ap()

    with tile.TileContext(nc) as tc, ExitStack() as ctx:
        const = ctx.enter_context(tc.tile_pool(name="const", bufs=1))
        xtp = ctx.enter_context(tc.tile_pool(name="xtp", bufs=2))
        ypool = ctx.enter_context(tc.tile_pool(name="yp", bufs=2))
        upool = ctx.enter_context(tc.tile_pool(name="up", bufs=3))
        opool = ctx.enter_context(tc.tile_pool(name="op", bufs=2))
        stat = ctx.enter_context(tc.tile_pool(name="stat", bufs=10))
        psz = ctx.enter_context(tc.tile_pool(name="psz", bufs=4, space="PSUM"))

        # --- small constants (vector memsets; no DMA dependency) ---
        ones_f32 = const.tile([1, P], F32)
        nc.vector.memset(ones_f32[:], 1.0)
        warm_mov = const.tile([1, OB], F32)
        nc.vector.memset(warm_mov[:], 0.0)
        eps_sb = const.tile([P, 1], F32)
        nc.vector.memset(eps_sb[:], EPS_EFF)
        ones_sb = const.tile([1, 2, P], F8)
        nc.vector.memset(ones_sb[:], 1.0)

        # --- PE warmup: keep the clock-ramp window busy while inputs stage.
        # fp32 matmuls stream at 4 cyc/row, so a few cover the staging time.
        warm_ps = psz.tile([P, D], F32, tag="z_ps")
        for w in range(5):
            nc.tensor.matmul(
                warm_ps[:, 0:OB], ones_f32[:], warm_mov[:], start=True, stop=True
            )

        # --- constants from HBM (scalar queue; k-split so matmul t only
        # waits for the k-pairs it reads) ---
        wt_sb = const.tile([P, KT, D], F8)  # [i_local, k, o]
        nc.sync.dma_start(out=wt_sb[:], in_=wth.rearrange("k p o -> p k o"))
        bias_sb = const.tile([1, 2, D], F8)
        nc.scalar.dma_start(out=bias_sb[:], in_=biash.unsqueeze(0))
        if not trivial_affine:
            gamma_f32 = const.tile([P, D], F32)
            nc.sync.dma_start(out=gamma_f32[:], in_=gamma.unsqueeze(0).to_broadcast([P, D]))
            gamma_sb = const.tile([P, D], F16)
            nc.scalar.activation(gamma_sb[:], gamma_f32[:], AF.Copy)
            beta_f32 = const.tile([P, D], F32)
            nc.sync.dma_start(out=beta_f32[:], in_=beta.unsqueeze(0).to_broadcast([P, D]))
            beta_sb = const.tile([P, D], F16)
            nc.scalar.activation(beta_sb[:], beta_f32[:], AF.Copy)

        nb = b_core // P

        x_chunks = {}
        y_chunks = {}
        o_chunks = {}

        def load_chunk(st, ksplit=False):
            xt_sb = xtp.tile([P, KT, ST], F8, name="xt_sb")  # [i_local, k, b]
            src = xt[st].rearrange("p (k b) -> p k b", k=KT)
            if ksplit:
                for t in range(KP):
                    nc.sync.dma_start(
                        out=xt_sb[:, 2 * t:2 * t + 2, :], in_=src[:, 2 * t:2 * t + 2, :]
                    )
            else:
                nc.sync.dma_start(out=xt_sb[:], in_=src)
            y_sb = ypool.tile([P, tpc, D], F16, name="y_sb")
            nc.sync.dma_start(out=y_sb[:], in_=yh[st])
            x_chunks[st] = xt_sb
            y_chunks[st] = y_sb
            o_chunks[st] = opool.tile([P, tpc, D], F16, name="o_sb")

        load_chunk(0)

        for bt in range(nb):
            st = bt // tpc
            j = bt % tpc
            last = bt == nb - 1
            if j == 0:
                if st >= 1:
                    # previous chunk's results are complete; store them first
                    # so later load issues on this queue don't block behind it
                    nc.sync.dma_start(out=out[st - 1], in_=o_chunks.pop(st - 1)[:])
                    x_chunks.pop(st - 1)
                    y_chunks.pop(st - 1)
                if st + 1 < nst:
                    load_chunk(st + 1)
            xt_sb = x_chunks[st]
            y_sb = y_chunks[st]
            o_sb = o_chunks[st]

            # --- matmuls: z'' = x @ W''.T + b'', DoubleRow fp8, PSUM fp32 ---
            z_ps = psz.tile([P, D], F32)
            if not last:
                for t in range(KP):
                    lhsT = xt_sb[:, 2 * t:2 * t + 2, bass.ts(j, P)]
                    for half in range(2):
                        nc.tensor.matmul(
                            z_ps[:, bass.ts(half, OB)],
                            lhsT,
                            wt_sb[:, 2 * t:2 * t + 2, bass.ts(half, OB)],
                            start=(t == 0),
                            stop=False,
                            perf_mode=DR,
                        )
                for half in range(2):
                    nc.tensor.matmul(
                        z_ps[:, bass.ts(half, OB)],
                        ones_sb[:],
                        bias_sb[:, :, bass.ts(half, OB)],
                        start=False,
                        stop=True,
                        perf_mode=DR,
                    )
            else:
                # last tile: half-major order so stats on half 0 overlap the
                # half-1 matmuls, shortening the end-of-kernel drain
                for half in range(2):
                    for t in range(KP):
                        nc.tensor.matmul(
                            z_ps[:, bass.ts(half, OB)],
                            xt_sb[:, 2 * t:2 * t + 2, bass.ts(j, P)],
                            wt_sb[:, 2 * t:2 * t + 2, bass.ts(half, OB)],
                            start=(t == 0),
                            stop=False,
                            perf_mode=DR,
                        )
                    nc.tensor.matmul(
                        z_ps[:, bass.ts(half, OB)],
                        ones_sb[:],
                        bias_sb[:, :, bass.ts(half, OB)],
                        start=False,
                        stop=True,
                        perf_mode=DR,
                    )
                nc.sync.dma_start(
                    out=out[st][:, 0:tpc - 1, :], in_=o_sb[:, 0:tpc - 1, :]
                )

            # --- stats: rstd = 1/sqrt(sum(z''^2)/D + eps) (mean is 0) ---
            std = stat.tile([P, 1], F32)
            if not last:
                ssq = stat.tile([P, 1], F32)
                z2_scr = upool.tile([P, D], F16)
                nc.scalar.activation(z2_scr[:], z_ps[:], AF.Square, accum_out=ssq[:])
                nc.scalar.activation(
                    std[:], ssq[:], AF.Sqrt, bias=eps_sb[:], scale=1.0 / D
                )
            else:
                ssqh = stat.tile([P, 2], F32)
                z2_scr = upool.tile([P, D], F16)
                for half in range(2):
                    nc.scalar.activation(
                        z2_scr[:, bass.ts(half, OB)],
                        z_ps[:, bass.ts(half, OB)],
                        AF.Square,
                        accum_out=ssqh[:, half:half + 1],
                    )
                ssq = stat.tile([P, 1], F32)
                nc.vector.tensor_add(
                    out=ssq[:], in0=ssqh[:, 0:1], in1=ssqh[:, 1:2]
                )
                nc.scalar.activation(
                    std[:], ssq[:], AF.Sqrt, bias=eps_sb[:], scale=1.0 / D
                )
            rstd = stat.tile([P, 1], F32)
            nc.vector.reciprocal(rstd[:], std[:])

            # --- normalize + residual: u = z''*rstd + y;  o = (u) * y ---
            u_sb = upool.tile([P, D], F16)
            if trivial_affine:
                if not last:
                    nc.vector.scalar_tensor_tensor(
                        out=u_sb[:], in0=z_ps[:], scalar=rstd[:], in1=y_sb[:, j, :],
                        op0=OP.mult, op1=OP.add,
                    )
                    # split the final multiply: VectorE low half, GpSimd high
                    nc.vector.tensor_mul(
                        out=o_sb[:, j, 0:384], in0=u_sb[:, 0:384], in1=y_sb[:, j, 0:384]
                    )
                    nc.gpsimd.tensor_mul(
                        out=o_sb[:, j, 384:D], in0=u_sb[:, 384:D], in1=y_sb[:, j, 384:D]
                    )
                else:
                    for half in range(2):
                        hs = bass.ts(half, OB)
                        nc.vector.scalar_tensor_tensor(
                            out=u_sb[:, hs], in0=z_ps[:, hs], scalar=rstd[:],
                            in1=y_sb[:, j, hs], op0=OP.mult, op1=OP.add,
                        )
                        nc.vector.tensor_mul(
                            out=o_sb[:, j, hs], in0=u_sb[:, hs], in1=y_sb[:, j, hs]
                        )
                        # store each half as soon as it is ready
                        nc.sync.dma_start(
                            out=out[st][:, j:j + 1, hs],
                            in_=o_sb[:, j:j + 1, hs],
                        )
            else:
                ln_sb = upool.tile([P, D], F16)
                nc.vector.scalar_tensor_tensor(
                    out=ln_sb[:], in0=z_ps[:], scalar=rstd[:], in1=gamma_sb[:],
                    op0=OP.mult, op1=OP.mult,
                )
                nc.vector.tensor_add(out=ln_sb[:], in0=ln_sb[:], in1=beta_sb[:])
                nc.vector.tensor_add(out=u_sb[:], in0=ln_sb[:], in1=y_sb[:, j, :])
                nc.vector.tensor_mul(
                    out=o_sb[:, j, :], in0=u_sb[:], in1=y_sb[:, j, :]
                )
                if last:
                    nc.sync.dma_start(
                        out=out[st][:, j:j + 1, :], in_=o_sb[:, j:j + 1, :]
                    )


    nc.finalize()
    _BUILD_CACHE[key] = nc
    return nc


def _run(nc, in_maps, **kwargs):
    return bass_utils.run_bass_kernel_spmd(
        nc, in_maps, core_ids=list(range(N_CORES)), **kwargs
    )


def _q8(a):
    return np.ascontiguousarray(a.astype(ml_dtypes.float8_e4m3))


def _prepare(x, y, weight, bias, gamma, beta):
    x = np.asarray(x, dtype=np.float32)
    y = np.ascontiguousarray(y, dtype=np.float32)
    weight = np.asarray(weight, dtype=np.float32)
    bias = np.asarray(bias, dtype=np.float32)
    gamma = np.asarray(gamma, dtype=np.float32)
    beta = np.asarray(beta, dtype=np.float32)

    B, IN = x.shape
    assert IN == D and weight.shape == (D, D) and y.shape == (B, D)
    assert B % (N_CORES * ST) == 0
    b_core = B // N_CORES
    nst = b_core // ST
    tpc = ST // P

    trivial = bool(np.all(gamma == 1.0)) and bool(np.all(beta == 0.0))
    nc = _build(b_core, trivial)

    # recenter: subtract each input-column's mean over outputs, so the matmul
    # result is already mean-free along the LayerNorm axis
    wcc = weight - weight.mean(axis=0, keepdims=True)
    bcc = bias - bias.mean()

    # W''.T packed: wth_prep[k, p, o] = W''.T[k*P + p, o], e4m3 at scale SW
    wth_prep = _q8((SW * wcc.T).reshape(KT, P, D))
    bias_prep = np.zeros((2, D), dtype=np.float32)
    bias_prep[0] = SCALE * bcc
    bias_prep = _q8(bias_prep)

    in_maps = []
    for c in range(N_CORES):
        xs = x[c * b_core:(c + 1) * b_core]
        # x8 packed: xt_prep[st, p, k, b_local] = x.T[k*P + p, st*ST + b_local]
        xt_prep = _q8(
            (SX * xs.T).reshape(KT, P, nst, ST).transpose(2, 1, 0, 3)
        ).reshape(nst, P, KT * ST)
        ys = y[c * b_core:(c + 1) * b_core].astype(np.float16)
        # y packed [st, p, j, o] = y[st*ST + j*P + p, o]
        y_prep = np.ascontiguousarray(
            ys.reshape(nst, tpc, P, D).transpose(0, 2, 1, 3)
        )
        m = {
            "xt": xt_prep,
            "yh": y_prep,
            "wth": wth_prep,
            "biash": bias_prep,
        }
        if not trivial:
            m["gamma"] = gamma
            m["beta"] = beta
        in_maps.append(m)
    return nc, in_maps


def kernel(x, y, weight, bias, gamma, beta):
    nc, in_maps = _prepare(x, y, weight, bias, gamma, beta)
    res = _run(nc, in_maps)
    b_core = x.shape[0] // N_CORES
    nst = b_core // ST
    tpc = ST // P
    outs = []
    for r in res.results:
        o = np.asarray(r["out"])  # [nst, P, tpc, D] fp16
        outs.append(o.transpose(0, 2, 1, 3).reshape(b_core, D))
    return np.concatenate(outs, axis=0).astype(np.float32)


# revision 9
# speedup vs baseline: 1.2049x; 1.1590x over previous
"""Fused Linear + LayerNorm + residual-multiply kernel for 8 Trainium2 cores.

Computes, for full inputs x[B,1024], y[B,1024], weight[1024,1024], bias, gamma, beta:
    z  = x @ weight.T + bias
    ln = (z - mean(z)) * rsqrt(var(z) + eps) * gamma + beta     (over last dim)
    out = (ln + y) * y

Data-parallel over the batch dim: each of the 8 NeuronCores processes B/8 rows;
weight/bias/gamma/beta are replicated. No cross-core communication.

Key host-side tricks:
  * Mean elimination: z - mean(z) == x @ W''.T + b'' where W'' subtracts each
    input-column's mean over outputs (W''[o,i] = W[o,i] - mean_o W[:,i]) and
    b'' = b - mean(b).  The device never computes the mean - only sum(z''^2).
  * fp8 DoubleRow matmuls: x and W'' are quantized host-side to e4m3 (scaled
    by 8 and 128; LayerNorm is scale-invariant so only eps is adjusted).
    DoubleRow packs 2 fp8 weights per PE cell; contraction runs over
    [128 partitions x 2 k-blocks] per matmul -> half the matmul count.

Per-core pipeline (b_core = B/8 rows, P=128, D=1024, 16 row-tiles):
  - PE: per tile 8 DoubleRow matmuls (4 k-pairs x 2 PSUM halves) + 2 DoubleRow
    bias matmuls; short fp32 warmup matmuls bridge the input-staging window so
    the clock ramp (HAM) reaches 2.4 GHz before the real matmuls.
  - ScalarE: Square activation with accum_out -> sum(z''^2); Sqrt(./D + eps).
  - VectorE: reciprocal -> rstd; fused scalar_tensor_tensor
    u = (z'' * rstd) + y straight out of PSUM; half of o = u * y.
  - GpSimd: other half of o = u * y  (SBUF-only fp16).
  - DMA: w''/bias on the scalar queue; x8/y loads and out stores on the sync
    queue, one chunk (512 rows) prefetched ahead, stores batched per chunk.
    The last tile runs a half-split low-latency chain to shorten the drain.
  - Output is fp16, widened to fp32 on the host.
"""

import numpy as np
import ml_dtypes
from contextlib import ExitStack

import concourse.bass as bass
import concourse.mybir as mybir
import concourse.tile as tile
from concourse import bacc, bass_utils


P = 128
D = 1024
KT = D // P          # 8 k-blocks of 128 over the contraction dim
KP = KT // 2         # 4 DoubleRow k-pairs
OB = 512             # o-block width (one PSUM bank of fp32)
ST = 512             # rows per super-chunk
N_CORES = 8
EPS = 1e-5

SX = 8.0             # host scale on x before e4m3 quantization
SW = 128.0           # host scale on W'' before e4m3 quantization
SCALE = SX * SW
EPS_EFF = EPS * SCALE * SCALE

F32 = mybir.dt.float32
F16 = mybir.dt.float16
F8 = mybir.dt.float8e4

AF = mybir.ActivationFunctionType
OP = mybir.AluOpType
DR = mybir.MatmulPerfMode.DoubleRow

_BUILD_CACHE = {}


def _build(b_core: int, trivial_affine: bool):
    key = (b_core, trivial_affine)
    if key in _BUILD_CACHE:
        return _BUILD_CACHE[key]

    nst = b_core // ST
    tpc = ST // P
    nc = bacc.Bacc("TRN2", debug=False, num_devices=N_CORES)

    # x8 packed as [super-chunk, partition, k * b_local] (contiguous/partition)
    xt = nc.dram_tensor("xt", [nst, P, KT * ST], F8, kind="ExternalInput").ap()
    # y packed row-tile-major: [chunk, partition, tile_in_chunk, D]
    yh = nc.dram_tensor("yh", [nst, P, tpc, D], F16, kind="ExternalInput").ap()
    # W''.T packed as [k, partition, o] (contiguous per partition per k-block)
    wth = nc.dram_tensor("wth", [KT, P, D], F8, kind="ExternalInput").ap()
    # bias row pair for DoubleRow: [2, D] with row 1 = 0
    biash = nc.dram_tensor("biash", [2, D], F8, kind="ExternalInput").ap()
    if not trivial_affine:
        gamma = nc.dram_tensor("gamma", [D], F32, kind="ExternalInput").ap()
        beta = nc.dram_tensor("beta", [D], F32, kind="ExternalInput").ap()
    out = nc.dram_tensor("out", [nst, P, tpc, D], F16, kind="ExternalOutput").ap()

    with tile.TileContext(nc) as tc, ExitStack() as ctx:
        const = ctx.enter_context(tc.tile_pool(name="const", bufs=1))
        xtp = ctx.enter_context(tc.tile_pool(name="xtp", bufs=3))
        ypool = ctx.enter_context(tc.tile_pool(name="yp", bufs=3))
        upool = ctx.enter_context(tc.tile_pool(name="up", bufs=3))
        opool = ctx.enter_context(tc.tile_pool(name="op", bufs=3))
        stat = ctx.enter_context(tc.tile_pool(name="stat", bufs=10))
        psz = ctx.enter_context(tc.tile_pool(name="psz", bufs=4, space="PSUM"))

        # --- small constants (vector memsets; no DMA dependency) ---
        ones_f32 = const.tile([1, P], F32)
        nc.vector.memset(ones_f32[:], 1.0)
        warm_mov = const.tile([1, 256], F32)
        nc.vector.memset(warm_mov[:], 0.0)
        eps_sb = const.tile([P, 1], F32)
        nc.vector.memset(eps_sb[:], EPS_EFF)
        ones_sb = const.tile([1, 2, P], F8)
        nc.vector.memset(ones_sb[:], 1.0)

        # --- PE warmup: keep the clock-ramp window busy while inputs stage.
        # fp32 matmuls stream at 4 cyc/row, so a few cover the staging time.
        warm_ps = psz.tile([P, D], F32, tag="z_ps")
        for w in range(2):
            nc.tensor.matmul(
                warm_ps[:, 0:256], ones_f32[:], warm_mov[:], start=True, stop=True
            )

        # --- constants from HBM (scalar queue; k-split so matmul t only
        # waits for the k-pairs it reads) ---
        wt_sb = const.tile([P, KT, D], F8)  # [i_local, k, o]
        wsrc = wth.rearrange("k p o -> p k o")
        nc.scalar.dma_start(out=wt_sb[:, 0:2, :], in_=wsrc[:, 0:2, :])
        nc.scalar.dma_start(out=wt_sb[:, 2:8, :], in_=wsrc[:, 2:8, :])
        bias_sb = const.tile([1, 2, D], F8)
        nc.scalar.dma_start(out=bias_sb[:], in_=biash.unsqueeze(0))
        if not trivial_affine:
            gamma_f32 = const.tile([P, D], F32)
            nc.sync.dma_start(out=gamma_f32[:], in_=gamma.unsqueeze(0).to_broadcast([P, D]))
            gamma_sb = const.tile([P, D], F16)
            nc.scalar.activation(gamma_sb[:], gamma_f32[:], AF.Copy)
            beta_f32 = const.tile([P, D], F32)
            nc.sync.dma_start(out=beta_f32[:], in_=beta.unsqueeze(0).to_broadcast([P, D]))
            beta_sb = const.tile([P, D], F16)
            nc.scalar.activation(beta_sb[:], beta_f32[:], AF.Copy)

        nb = b_core // P

        x_chunks = {}
        y_chunks = {}
        o_chunks = {}

        def load_chunk(st, ksplit=False):
            xt_sb = xtp.tile([P, KT, ST], F8, name="xt_sb")  # [i_local, k, b]
            src = xt[st].rearrange("p (k b) -> p k b", k=KT)
            if ksplit:
                for t in range(KP):
                    nc.sync.dma_start(
                        out=xt_sb[:, 2 * t:2 * t + 2, :], in_=src[:, 2 * t:2 * t + 2, :]
                    )
            else:
                nc.sync.dma_start(out=xt_sb[:], in_=src)
            y_sb = ypool.tile([P, tpc, D], F16, name="y_sb")
            nc.sync.dma_start(out=y_sb[:], in_=yh[st])
            x_chunks[st] = xt_sb
            y_chunks[st] = y_sb
            o_chunks[st] = opool.tile([P, tpc, D], F16, name="o_sb")

        load_chunk(0, ksplit=True)

        for bt in range(nb):
            st = bt // tpc
            j = bt % tpc
            last = bt == nb - 1
            if j == 0:
                if st + 1 < nst:
                    load_chunk(st + 1)
                if st >= 1:
                    nc.sync.dma_start(out=out[st - 1], in_=o_chunks.pop(st - 1)[:])
                    x_chunks.pop(st - 1)
                    y_chunks.pop(st - 1)
            xt_sb = x_chunks[st]
            y_sb = y_chunks[st]
            o_sb = o_chunks[st]

            # --- matmuls: z'' = x @ W''.T + b'', DoubleRow fp8, PSUM fp32 ---
            z_ps = psz.tile([P, D], F32)
            if not last:
                for t in range(KP):
                    lhsT = xt_sb[:, 2 * t:2 * t + 2, bass.ts(j, P)]
                    for half in range(2):
                        nc.tensor.matmul(
                            z_ps[:, bass.ts(half, OB)],
                            lhsT,
                            wt_sb[:, 2 * t:2 * t + 2, bass.ts(half, OB)],
                            start=(t == 0),
                            stop=False,
                            perf_mode=DR,
                        )
                for half in range(2):
                    nc.tensor.matmul(
                        z_ps[:, bass.ts(half, OB)],
                        ones_sb[:],
                        bias_sb[:, :, bass.ts(half, OB)],
                        start=False,
                        stop=True,
                        perf_mode=DR,
                    )
            else:
                # last tile: half-major order so stats on half 0 overlap the
                # half-1 matmuls, shortening the end-of-kernel drain
                for half in range(2):
                    for t in range(KP):
                        nc.tensor.matmul(
                            z_ps[:, bass.ts(half, OB)],
                            xt_sb[:, 2 * t:2 * t + 2, bass.ts(j, P)],
                            wt_sb[:, 2 * t:2 * t + 2, bass.ts(half, OB)],
                            start=(t == 0),
                            stop=False,
                            perf_mode=DR,
                        )
                    nc.tensor.matmul(
                        z_ps[:, bass.ts(half, OB)],
                        ones_sb[:],
                        bias_sb[:, :, bass.ts(half, OB)],
                        start=False,
                        stop=True,
                        perf_mode=DR,
                    )
                nc.sync.dma_start(
                    out=out[st][:, 0:tpc - 1, :], in_=o_sb[:, 0:tpc - 1, :]
                )

            # --- stats: rstd = 1/sqrt(sum(z''^2)/D + eps) (mean is 0) ---
            std = stat.tile([P, 1], F32)
            if not last:
                ssq = stat.tile([P, 1], F32)
                z2_scr = upool.tile([P, D], F16)
                nc.scalar.activation(z2_scr[:], z_ps[:], AF.Square, accum_out=ssq[:])
                nc.scalar.activation(
                    std[:], ssq[:], AF.Sqrt, bias=eps_sb[:], scale=1.0 / D
                )
            else:
                ssqh = stat.tile([P, 2], F32)
                z2_scr = upool.tile([P, D], F16)
                for half in range(2):
                    nc.scalar.activation(
                        z2_scr[:, bass.ts(half, OB)],
                        z_ps[:, bass.ts(half, OB)],
                        AF.Square,
                        accum_out=ssqh[:, half:half + 1],
                    )
                ssq = stat.tile([P, 1], F32)
                nc.vector.tensor_add(
                    out=ssq[:], in0=ssqh[:, 0:1], in1=ssqh[:, 1:2]
                )
                nc.scalar.activation(
                    std[:], ssq[:], AF.Sqrt, bias=eps_sb[:], scale=1.0 / D
                )
            rstd = stat.tile([P, 1], F32)
            nc.vector.reciprocal(rstd[:], std[:])

            # --- normalize + residual: u = z''*rstd + y;  o = (u) * y ---
            u_sb = upool.tile([P, D], F16)
            if trivial_affine:
                if not last:
                    nc.vector.scalar_tensor_tensor(
                        out=u_sb[:], in0=z_ps[:], scalar=rstd[:], in1=y_sb[:, j, :],
                        op0=OP.mult, op1=OP.add,
                    )
                    # split the final multiply: VectorE low half, GpSimd high
                    nc.vector.tensor_mul(
                        out=o_sb[:, j, 0:384], in0=u_sb[:, 0:384], in1=y_sb[:, j, 0:384]
                    )
                    nc.gpsimd.tensor_mul(
                        out=o_sb[:, j, 384:D], in0=u_sb[:, 384:D], in1=y_sb[:, j, 384:D]
                    )
                else:
                    for half in range(2):
                        hs = bass.ts(half, OB)
                        nc.vector.scalar_tensor_tensor(
                            out=u_sb[:, hs], in0=z_ps[:, hs], scalar=rstd[:],
                            in1=y_sb[:, j, hs], op0=OP.mult, op1=OP.add,
                        )
                        nc.vector.tensor_mul(
                            out=o_sb[:, j, hs], in0=u_sb[:, hs], in1=y_sb[:, j, hs]
                        )
                        # store each half as soon as it is ready
                        nc.sync.dma_start(
                            out=out[st][:, j:j + 1, hs],
                            in_=o_sb[:, j:j + 1, hs],
                        )
            else:
                ln_sb = upool.tile([P, D], F16)
                nc.vector.scalar_tensor_tensor(
                    out=ln_sb[:], in0=z_ps[:], scalar=rstd[:], in1=gamma_sb[:],
                    op0=OP.mult, op1=OP.mult,
                )
                nc.vector.tensor_add(out=ln_sb[:], in0=ln_sb[:], in1=beta_sb[:])
                nc.vector.tensor_add(out=u_sb[:], in0=ln_sb[:], in1=y_sb[:, j, :])
                nc.vector.tensor_mul(
                    out=o_sb[:, j, :], in0=u_sb[:], in1=y_sb[:, j, :]
                )
                if last:
                    nc.sync.dma_start(
                        out=out[st][:, j:j + 1, :], in_=o_sb[:, j:j + 1, :]
                    )


    nc.finalize()
    _BUILD_CACHE[key] = nc
    return nc


def _run(nc, in_maps, **kwargs):
    return bass_utils.run_bass_kernel_spmd(
        nc, in_maps, core_ids=list(range(N_CORES)), **kwargs
    )


def _q8(a):
    return np.ascontiguousarray(a.astype(ml_dtypes.float8_e4m3))


def _prepare(x, y, weight, bias, gamma, beta):
    x = np.asarray(x, dtype=np.float32)
    y = np.ascontiguousarray(y, dtype=np.float32)
    weight = np.asarray(weight, dtype=np.float32)
    bias = np.asarray(bias, dtype=np.float32)
    gamma = np.asarray(gamma, dtype=np.float32)
    beta = np.asarray(beta, dtype=np.float32)

    B, IN = x.shape
    assert IN == D and weight.shape == (D, D) and y.shape == (B, D)
    assert B % (N_CORES * ST) == 0
    b_core = B // N_CORES
    nst = b_core // ST
    tpc = ST // P

    trivial = bool(np.all(gamma == 1.0)) and bool(np.all(beta == 0.0))
    nc = _build(b_core, trivial)

    # recenter: subtract each input-column's mean over outputs, so the matmul
    # result is already mean-free along the LayerNorm axis
    wcc = weight - weight.mean(axis=0, keepdims=True)
    bcc = bias - bias.mean()

    # W''.T packed: wth_prep[k, p, o] = W''.T[k*P + p, o], e4m3 at scale SW
    wth_prep = _q8((SW * wcc.T).reshape(KT, P, D))
    bias_prep = np.zeros((2, D), dtype=np.float32)
    bias_prep[0] = SCALE * bcc
    bias_prep = _q8(bias_prep)

    in_maps = []
    for c in range(N_CORES):
        xs = x[c * b_core:(c + 1) * b_core]
        # x8 packed: xt_prep[st, p, k, b_local] = x.T[k*P + p, st*ST + b_local]
        xt_prep = _q8(
            (SX * xs.T).reshape(KT, P, nst, ST).transpose(2, 1, 0, 3)
        ).reshape(nst, P, KT * ST)
        ys = y[c * b_core:(c + 1) * b_core].astype(np.float16)
        # y packed [st, p, j, o] = y[st*ST + j*P + p, o]
        y_prep = np.ascontiguousarray(
            ys.reshape(nst, tpc, P, D).transpose(0, 2, 1, 3)
        )
        m = {
            "xt": xt_prep,
            "yh": y_prep,
            "wth": wth_prep,
            "biash": bias_prep,
        }
        if not trivial:
            m["gamma"] = gamma
            m["beta"] = beta
        in_maps.append(m)
    return nc, in_maps


def kernel(x, y, weight, bias, gamma, beta):
    nc, in_maps = _prepare(x, y, weight, bias, gamma, beta)
    res = _run(nc, in_maps)
    b_core = x.shape[0] // N_CORES
    nst = b_core // ST
    tpc = ST // P
    outs = []
    for r in res.results:
        o = np.asarray(r["out"])  # [nst, P, tpc, D] fp16
        outs.append(o.transpose(0, 2, 1, 3).reshape(b_core, D))
    return np.concatenate(outs, axis=0).astype(np.float32)
